# revision 25
# baseline (speedup 1.0000x reference)
"""Trainium2 Bass kernel for a recurrent adaptive-LIF SNN.

Network (per reference):
    B=1024, T=100, n_in=120, h1=512, h2=256, n_out=35
    per step t:
        cur1 = x_t @ W1.T + s1 @ Wrec.T
        a1' = rho1*a1 + (1-rho1)*s1
        v1' = alpha1*v1*(1-s1) + (1-alpha1)*cur1
        s1' = (v1' - (1 + beta_a1*a1') > 0)
        cur2 = s1' @ W2.T ; same LIF for layer 2
        vo' = beta_out*vo + (1-beta_out)*(s2' @ W3.T)
    out = mean_t vo(t)

Sharding: data-parallel over batch across 8 cores (128 batch/core),
weights replicated; the sequential T loop is local per core.

Layout: feature-major [feature -> partitions, batch -> free].

Layer 1 (shifted P1 := v1'-1; scaled copy cp1 := P1/cb1, cb1=beta_a1(1-rho1)):
    p1psum = W1a@[x;1] + WrecF@s1 + a1*cb1*(cp1_prev - r1_prev)   (diag matmuls)
    cp1    = ACT(p1psum, scale=1/cb1)        [= P1/cb1]
    u1'    = rho1*u1 + s1                    [STT, GPSIMD]
    s1'    = (u1' < cp1)                     [DVE is_lt  <=> cb1*u1' < P1]
    r1'    = s1' * cp1                       [DVE mult]
  since -a1*q1 = -a1*(s1-1)*P1 = a1*cb1*(cp1 - r1).  W1a has a const row
  (a1-1); WrecF = ((1-a1)Wrec).T - a1*I runs as fp8e4 DoubleRow with a 2^7
  weight scale balanced by fp8 spike mirrors sf = 2^-7*s1.
  Init: cp1_init = -1/cb1, r1_init = 0  (reproduces q1_init = 1).

Layer 2 (scaled Pt2 := v2/cb2; psum tracks Pt2 - a2/cb2):
    p2psum = W2s@s1' + a2*(cp2_prev - r2_prev) - (a2/cb2)*s2_prev  (diags)
    cp2    = ACT(p2psum, bias=(a2-1)/cb2)    [= Pt2 - 1/cb2]
    W2t'   = rho2*W2t + s2                   [STT, DVE]
    s2'    = (W2t' < cp2)                    [DVE]
    r2'    = s2' * cp2                       [GPSIMD mult]
  W2s = 2^7*((1-a2)/cb2 * W2).T as fp8e4 DoubleRow over the s1 mirrors.
  Init r2_init = 1/cb2, cp2_init = 0.

Output (closed form, no integrator state, no epilogue GEMM):
    out = sum_t (c_t*W3) @ s2(t),  c_t = (1 - beta_out^(T-t))/T
  accumulated directly in a persistent PSUM region with per-step
  pre-scaled stationary weights.

Engine split per step (cost-model balanced; the HW ISA rejects
TensorScalarPtr on Pool, so the STT-ish updates stay on DVE):
  PE    : W1 prefetch, fp8-DR Wrec + W2, diag compensations, out accumulation
  ACT   : the three scaled PSUM->SBUF copies (cp1 a/b halves, cp2)
  DVE   : spike compares, fp8 mirrors, u1 update as tensor_scalar(4x mode)
          + add(2x mode) [cheaper than the modeless STT], W2t STT
  GPSIMD: the reset products r = s*cp (full-step slack hides Q7 latency)
"""

import sys
import numpy as np

sys.path.insert(0, "/opt/trn_rl_repo")

import ml_dtypes

bf16 = ml_dtypes.bfloat16
f8e4 = ml_dtypes.float8_e4m3

# Problem constants (hardcoded per contract)
B, T, N_IN, H1, H2, N_OUT = 1024, 100, 120, 512, 256, 35
N_CORES = 8
BC = B // N_CORES  # 128 batch per core
C1 = H1 // 128     # 4 feature chunks, layer 1
C2 = H2 // 128     # 2 feature chunks, layer 2
K1 = N_IN + 1      # x augmented with a constant-one row

FP8_SHIFT = 7      # wrec8/w28 = 2^7 * W, fp8 spike mirror value = 2^-7

_CACHE = {}


def _build(alpha1, rho1, beta_a1, alpha2, rho2, beta_a2, beta_out):
    import concourse.bacc as bacc
    import concourse.mybir as mybir
    import concourse.tile as tile
    from concourse.alu_op_type import AluOpType

    fp32 = mybir.dt.float32
    bft = mybir.dt.bfloat16
    f8t = mybir.dt.float8e4
    A = AluOpType
    IDENT = mybir.ActivationFunctionType.Identity
    DR = mybir.MatmulPerfMode.DoubleRow

    a1 = float(alpha1)
    a2 = float(alpha2)
    cb1 = float(beta_a1 * (1.0 - rho1))
    cb2 = float(beta_a2 * (1.0 - rho2))
    sfp8 = float(2.0 ** -FP8_SHIFT)

    nc = bacc.Bacc()

    x_d = nc.declare_dram_parameter("x", [K1, T, BC], bft, isOutput=False)
    w1_d = nc.declare_dram_parameter("w1s", [K1, C1, 128], bft, isOutput=False)
    wr_d = nc.declare_dram_parameter(
        "wrec8", [128, 2, C1, 2, 128], f8t, isOutput=False
    )
    w2_d = nc.declare_dram_parameter(
        "w28", [128, 2, C2, 2, 128], f8t, isOutput=False
    )
    w3_d = nc.declare_dram_parameter("w3c", [128, C2, T, N_OUT], bft, isOutput=False)
    # diagonal compensation matrices
    dg_d = nc.declare_dram_parameter("diags", [128, 7, 128], bft, isOutput=False)
    out_d = nc.declare_dram_parameter("out", [N_OUT, BC], fp32, isOutput=True)

    XCH = 10  # x preload chunks
    TP = T // XCH

    with tile.TileContext(nc) as tc:
        with (
            tc.tile_pool(name="wpool", bufs=1) as wpool,
            tc.tile_pool(name="xpool", bufs=1) as xpool,
            tc.tile_pool(name="st1", bufs=12) as st1,
            tc.tile_pool(name="st2", bufs=8) as st2,
            tc.tile_pool(name="cp", bufs=12) as cpp,
            tc.tile_pool(name="sf", bufs=8) as sfp,
            tc.tile_pool(name="ps1a", bufs=2, space="PSUM") as ps1a,
            tc.tile_pool(name="ps1b", bufs=2, space="PSUM") as ps1b,
            tc.tile_pool(name="ps2", bufs=1, space="PSUM") as ps2,
            tc.tile_pool(name="psW", bufs=1, space="PSUM") as psW,
            tc.tile_pool(name="psO", bufs=1, space="PSUM") as psO,
        ):
            # ---- resident weights ----
            w1_s = wpool.tile([K1, C1, 128], bft, tag="w1")
            nc.sync.dma_start(w1_s[:], w1_d[:])
            wr_s = wpool.tile([128, 2, C1, 2, 128], f8t, tag="wr")
            nc.sync.dma_start(wr_s[:], wr_d[:])
            w2_s = wpool.tile([128, 2, C2, 2, 128], f8t, tag="w2")
            nc.sync.dma_start(w2_s[:], w2_d[:])
            w3_s = wpool.tile([128, C2, T, N_OUT], bft, tag="w3")
            nc.sync.dma_start(w3_s[:], w3_d[:])
            dg_s = wpool.tile([128, 7, 128], bft, tag="dg")
            nc.sync.dma_start(dg_s[:], dg_d[:])
            # diag slots: 0:+a1*cb1  1:-a1*cb1  2:+a2  3:-a2  4:-a2/cb2
            #             5:rho2     6:identity
            PD1, ND1, PD2, ND2, SD2, RD2, ID = (dg_s[:, i, :] for i in range(7))
            # per-partition bias column for the layer-2 scaled copy
            b2_s = wpool.tile([128, 1], fp32, tag="b2")
            nc.vector.memset(b2_s[:], (a2 - 1.0) / cb2)

            # ---- x preload in chunks ----
            x_tiles = []
            for i in range(XCH):
                xt = xpool.tile([K1, TP, BC], bft, tag=f"x{i}")
                nc.sync.dma_start(xt[:], x_d[:, i * TP : (i + 1) * TP, :])
                x_tiles.append(xt)

            # ---- initial states ----
            s1 = st1.tile([128, C1, BC], bft, tag="s1")
            nc.vector.memset(s1[:], 0.0)
            u1 = st1.tile([128, C1, BC], bft, tag="u1")
            nc.vector.memset(u1[:], 0.0)
            r1 = st1.tile([128, C1, BC], bft, tag="r1")
            nc.vector.memset(r1[:], 0.0)
            cpa = cpp.tile([128, 2, BC], bft, tag="cpa")
            nc.vector.memset(cpa[:], -1.0 / cb1)
            cpb = cpp.tile([128, 2, BC], bft, tag="cpb")
            nc.vector.memset(cpb[:], -1.0 / cb1)
            sfa = sfp.tile([128, 2, BC], f8t, tag="sfa")
            nc.vector.memset(sfa[:], 0.0)
            sfb = sfp.tile([128, 2, BC], f8t, tag="sfb")
            nc.vector.memset(sfb[:], 0.0)
            w2t = st2.tile([128, C2, BC], bft, tag="w2t")
            nc.vector.memset(w2t[:], 0.0)
            s2 = st2.tile([128, C2, BC], bft, tag="s2")
            nc.vector.memset(s2[:], 0.0)
            r2 = st2.tile([128, C2, BC], bft, tag="r2")
            nc.vector.memset(r2[:], 1.0 / cb2)
            cp2 = cpp.tile([128, C2, BC], bft, tag="cp2")
            nc.vector.memset(cp2[:], 0.0)

            # chunk -> s1n/u1n/r1 tile slot.  Bank A (p1a) holds chunks 0,2
            # (slots 0,1 -> copy cpa); bank B (p1b) holds chunks 1,3 (slots
            # 2,3 -> copy cpb).  This pairing lets each bank's reset half (r)
            # close its own recurrence without waiting for the other.
            CS = {0: 0, 2: 1, 1: 2, 3: 3}
            # bank -> (chunk in slot 0, chunk in slot 1)
            BANK_CHUNKS = {0: (0, 2), 1: (1, 3)}

            # ---- prologue: open the p1 banks for t=0.  Each bank is a
            # single accumulation group over [128, 2, BC]: the W1 prefetch
            # (slot-0 slice) carries start=True; slice writers use
            # skip_group_check; the full-tile ND1 carries stop.
            p1a = ps1a.tile([128, 2, BC], fp32, tag="p1a")
            p1b = ps1b.tile([128, 2, BC], fp32, tag="p1b")
            for m, ph in ((0, p1a), (1, p1b)):
                nc.tensor.matmul(
                    ph[:, 0, :], w1_s[:, m, :], x_tiles[0][:, 0, :],
                    start=True, stop=False,
                )

            def l1_bank(bank, ph, xsl_t):
                # close bank `bank` for this step: W1 slot-1 slice, full-tile
                # PD1 compensation, the four DR wrec slices, full-tile ND1
                # (stop).  All one PSUM group; no intra-bank region ordering.
                c_lo, c_hi = BANK_CHUNKS[bank]
                cph = cpa if bank == 0 else cpb
                rsl = r1[:, 2 * bank : 2 * bank + 2, :]
                nc.tensor.matmul(ph[:, 1, :], w1_s[:, c_hi, :], xsl_t,
                                 start=False, stop=False, skip_group_check=True)
                nc.tensor.matmul(ph[:], PD1, cph[:], start=False, stop=False,
                                 skip_group_check=True)
                # ND1 early (its r input is ready from the previous step);
                # the group stop rides the last sfb-dependent DR slice so the
                # bank closes right behind the late mirror
                nc.tensor.matmul(ph[:], ND1, rsl, start=False, stop=False,
                                 skip_group_check=True)
                for sl, m in ((0, c_lo), (1, c_hi)):
                    nc.tensor.matmul(ph[:, sl, :], wr_s[:, 0, m, :, :], sfa[:],
                                     start=False, stop=False, perf_mode=DR,
                                     skip_group_check=True)
                for sl, m in ((0, c_lo), (1, c_hi)):
                    nc.tensor.matmul(ph[:, sl, :], wr_s[:, 1, m, :, :], sfb[:],
                                     start=False, stop=(sl == 1), perf_mode=DR,
                                     skip_group_check=True)

            def l2_state(tau):
                # L2 state updates for step tau.  w2t already holds W~2(tau)
                # (computed on PE+ACT during step tau).  The compare/reset run
                # on DVE right behind cp2; W~2(tau+1) = rho2*W~2 + s2 runs as
                # two full-tile diag matmuls + one ACT copy (a full step of
                # slack).
                nonlocal s2, r2
                s2n = st2.tile([128, C2, BC], bft, tag="s2")
                r2n = st2.tile([128, C2, BC], bft, tag="r2")
                pw = None
                nc.vector.tensor_tensor(s2n[:], w2t[:], cp2[:], A.is_lt)
                if tau < T - 1:
                    nc.gpsimd.tensor_tensor(r2n[:], s2n[:], cp2[:], A.mult)
                # output accumulation + the W~2 recurrence have nearly a full
                # step of slack -> deprioritized so they never clog the PE
                # queue ahead of cycle-critical work
                with tc.high_priority(offset=-1_000_000):
                    # out += (cw_tau * W3)[k] @ s2n[k]
                    for k in range(C2):
                        nc.tensor.matmul(
                            out_ps[:], w3_s[:, k, tau, :], s2n[:, k, :],
                            start=(tau == 0 and k == 0),
                            stop=(tau == T - 1 and k == C2 - 1),
                            skip_group_check=True,
                        )
                if tau < T - 1:
                    # normal priority: pw must close early so the w2tn copy
                    # runs in ACT's mid-step idle window
                    pw = psW.tile([128, C2, BC], fp32, tag="pw")
                    nc.tensor.matmul(pw[:], RD2, w2t[:], start=True, stop=False)
                    nc.tensor.matmul(pw[:], ID, s2n[:], start=False, stop=True)
                s2, r2 = s2n, r2n
                return pw if tau < T - 1 else None

            out_ps = psO.tile([N_OUT, BC], fp32, tag="ops")

            u1n = u1
            for t in range(T):
                # ---- L2 state updates for step t-1 (cmp2 heads the DVE
                # queue: its input cp2(t-1) is ready at step start) ----
                pw_t = l2_state(t - 1) if t >= 1 else None

                # ---- u1 adaptation, part 1: u1t(t+1) = rho1*u1n(t); input
                # ready since the tail of step t-1, fills early-step DVE idle
                u1t_n = st1.tile([128, C1, BC], bft, tag="u1t")
                nc.vector.tensor_scalar(u1t_n[:], u1n[:], float(rho1), None, A.mult)

                # ---- finish p1(t): single accumulation group per bank ----
                xsl_t = x_tiles[t // TP][:, t % TP, :]
                with tc.high_priority(offset=500_000):
                    l1_bank(0, p1a, xsl_t)
                    l1_bank(1, p1b, xsl_t)

                # ---- scaled PSUM -> SBUF copies of P1 ----
                cpa_n = cpp.tile([128, 2, BC], bft, tag="cpa")
                nc.scalar.activation(cpa_n[:], p1a[:], IDENT, scale=1.0 / cb1)
                cpb_n = cpp.tile([128, 2, BC], bft, tag="cpb")
                nc.scalar.activation(cpb_n[:], p1b[:], IDENT, scale=1.0 / cb1)

                # ---- layer-1 spike compares + fp8 mirrors on DVE (per bank
                # half so each bank's recurrence closes independently).
                # Reset products: bank A's goes to GPSIMD (it closes early,
                # so Pool's latency hides); bank B's stays on DVE right
                # behind its mirror so the B-side cycle never waits on Pool.
                s1n = st1.tile([128, C1, BC], bft, tag="s1")
                r1n = st1.tile([128, C1, BC], bft, tag="r1")
                sfa_n = sfp.tile([128, 2, BC], f8t, tag="sfa")
                sfb_n = sfp.tile([128, 2, BC], f8t, tag="sfb")
                nc.vector.tensor_tensor(
                    s1n[:, 0:2, :], u1n[:, 0:2, :], cpa_n[:], A.is_lt
                )
                nc.vector.tensor_scalar(sfa_n[:], s1n[:, 0:2, :], sfp8, None, A.mult)
                if t + 1 < T:
                    nc.gpsimd.tensor_tensor(
                        r1n[:, 0:2, :], s1n[:, 0:2, :], cpa_n[:], A.mult
                    )
                nc.vector.tensor_tensor(
                    s1n[:, 2:4, :], u1n[:, 2:4, :], cpb_n[:], A.is_lt
                )
                nc.vector.tensor_scalar(sfb_n[:], s1n[:, 2:4, :], sfp8, None, A.mult)
                if t + 1 < T:
                    nc.vector.tensor_tensor(
                        r1n[:, 2:4, :], s1n[:, 2:4, :], cpb_n[:], A.mult
                    )

                # ---- u1 adaptation, part 2: u1n(t+1) = u1t + s1n(t) ----
                u1n_n = st1.tile([128, C1, BC], bft, tag="u1")
                nc.vector.tensor_tensor(u1n_n[:], u1t_n[:], s1n[:], A.add)

                # ---- W1 prefetch into next step's p1 banks (slot-0 slice
                # carries the group start) ----
                if t + 1 < T:
                    p1a_n = ps1a.tile([128, 2, BC], fp32, tag="p1a")
                    p1b_n = ps1b.tile([128, 2, BC], fp32, tag="p1b")
                    xsl = x_tiles[(t + 1) // TP][:, (t + 1) % TP, :]
                    nc.tensor.matmul(p1a_n[:, 0, :], w1_s[:, 0, :], xsl,
                                     start=True, stop=False)
                    nc.tensor.matmul(p1b_n[:, 0, :], w1_s[:, 1, :], xsl,
                                     start=True, stop=False)
                else:
                    p1a_n = p1b_n = None

                # ---- layer 2: p2 = W2s@s1' (fp8 DR over the mirrors) +
                # diag compensations; one accumulation group, full-tile
                # diags + DR slice writes ----
                p2 = ps2.tile([128, C2, BC], fp32, tag="p2")
                nc.tensor.matmul(p2[:], PD2, cp2[:], start=True, stop=False)
                nc.tensor.matmul(p2[:], SD2, s2[:], start=False, stop=False,
                                 skip_group_check=True)
                for m in range(C2):
                    nc.tensor.matmul(p2[:, m, :], w2_s[:, 0, m, :, :], sfa_n[:],
                                     start=False, stop=False, perf_mode=DR,
                                     skip_group_check=True)
                    nc.tensor.matmul(p2[:, m, :], w2_s[:, 1, m, :, :], sfb_n[:],
                                     start=False, stop=False, perf_mode=DR,
                                     skip_group_check=True)
                nc.tensor.matmul(p2[:], ND2, r2[:], start=False, stop=True,
                                 skip_group_check=True)

                # ---- tail copies: W~2 then cp2 on ACT, after the
                # cycle-critical cp1 copies in queue order ----
                if pw_t is not None:
                    w2tn = cpp.tile([128, C2, BC], bft, tag="w2t")
                    nc.scalar.activation(w2tn[:], pw_t[:], IDENT)
                    w2t = w2tn
                cp2_n = cpp.tile([128, C2, BC], bft, tag="cp2")
                nc.scalar.activation(cp2_n[:], p2[:], IDENT, bias=b2_s[:])

                s1, r1, u1n = s1n, r1n, u1n_n
                cpa, cpb, sfa, sfb = cpa_n, cpb_n, sfa_n, sfb_n
                cp2 = cp2_n
                p1a, p1b = p1a_n, p1b_n

            # ---- epilogue: last L2 state (includes final out term), then
            # writeback ----
            l2_state(T - 1)
            outf = cpp.tile([N_OUT, BC], fp32, tag="outf")
            nc.vector.tensor_scalar(outf[:], out_ps[:], 1.0, None, A.mult)
            nc.sync.dma_start(out_d[:], outf[:])

    nc.compile()
    return nc


def _prep_inputs(x, W1, Wrec, W2, W3, alpha1, rho1, beta_a1, alpha2, rho2, beta_a2, beta_out):
    a1 = float(np.asarray(alpha1).reshape(-1)[0])
    a2 = float(np.asarray(alpha2).reshape(-1)[0])
    r1 = float(np.asarray(rho1).reshape(-1)[0])
    r2 = float(np.asarray(rho2).reshape(-1)[0])
    ba1 = float(np.asarray(beta_a1).reshape(-1)[0])
    ba2 = float(np.asarray(beta_a2).reshape(-1)[0])
    bo = float(np.asarray(beta_out).reshape(-1)[0])
    cb1 = ba1 * (1.0 - r1)
    cb2 = ba2 * (1.0 - r2)

    w1s = ((1.0 - np.asarray(alpha1, np.float32)[:, None]) * np.asarray(W1, np.float32)).T
    wrs = ((1.0 - np.asarray(alpha1, np.float32)[:, None]) * np.asarray(Wrec, np.float32)).T
    w2s = (((1.0 - np.asarray(alpha2, np.float32)[:, None]) / cb2) * np.asarray(W2, np.float32)).T
    w3s = np.asarray(W3, np.float32).T

    # layer-1 shift folds:  WrecF = wrs - a1*I ; W1 gains const row (a1-1)
    wrs = wrs - a1 * np.eye(H1, dtype=np.float32)
    w1aug = np.concatenate(
        [w1s, np.full((1, H1), a1 - 1.0, np.float32)], axis=0
    )  # [121, 512]

    w1_a = np.ascontiguousarray(w1aug.reshape(K1, C1, 128)).astype(bf16)

    def pack_dr(w, cout):
        # w [H1, cout*128]: dr[part, pair, m, i, col] =
        #   2^7 * w[k(pair,i)*128+part, m*128+col], k(pair,i) = pair + 2*i
        # so DR pair 0 eats chunks (0,2) = spike mirror sfa and pair 1 eats
        # (1,3) = sfb.
        w4 = w.reshape(C1, 128, cout, 128)  # [k, part, m, col]
        w8 = (2.0 ** FP8_SHIFT) * w4.reshape(2, 2, 128, cout, 128)  # [i, pair, ...]
        return np.ascontiguousarray(w8.transpose(2, 1, 3, 0, 4)).astype(f8e4)

    wr_a = pack_dr(wrs, C1)
    w2_a = pack_dr(w2s, C2)

    # per-step output weights c_t = (1 - beta^(T-t))/T folded into W3
    cw = np.array([(1.0 - bo ** (T - t)) / T for t in range(T)], np.float32)
    w3_a = np.ascontiguousarray(
        w3s.reshape(C2, 128, N_OUT)[:, None, :, :].transpose(1, 0, 2, 3)
        * cw[:, None, None, None]
    )  # [T, C2, 128, N_OUT]
    w3_a = np.ascontiguousarray(w3_a.transpose(2, 1, 0, 3)).astype(bf16)
    # -> [128, C2, T, N_OUT]

    eye = np.eye(128, dtype=np.float32)
    diags = np.stack([
        (a1 * cb1) * eye,     # PD1
        (-a1 * cb1) * eye,    # ND1
        a2 * eye,             # PD2
        (-a2) * eye,          # ND2
        (-a2 / cb2) * eye,    # SD2
        r2 * eye,             # RD2
        eye,                  # ID
    ], axis=1).astype(bf16)   # [128, 7, 128]

    shared = dict(w1s=w1_a, wrec8=wr_a, w28=w2_a, w3c=w3_a, diags=diags)
    in_maps = []
    for c in range(N_CORES):
        xc = np.asarray(x[c * BC : (c + 1) * BC], np.float32)  # [BC, T, N_IN]
        xfm = xc.transpose(2, 1, 0)  # [N_IN, T, BC]
        xaug = np.concatenate([xfm, np.ones((1, T, BC), np.float32)], axis=0)
        in_maps.append(dict(x=np.ascontiguousarray(xaug).astype(bf16), **shared))
    return in_maps


def kernel(
    x, W1, Wrec, W2, W3,
    alpha1, rho1, beta_a1, alpha2, rho2, beta_a2, beta_out,
    _trace=False,
):
    from concourse.bass_utils import run_bass_kernel_spmd

    key = "nc"
    if key not in _CACHE:
        _CACHE[key] = _build(
            float(np.asarray(alpha1).reshape(-1)[0]),
            float(np.asarray(rho1).reshape(-1)[0]),
            float(np.asarray(beta_a1).reshape(-1)[0]),
            float(np.asarray(alpha2).reshape(-1)[0]),
            float(np.asarray(rho2).reshape(-1)[0]),
            float(np.asarray(beta_a2).reshape(-1)[0]),
            float(np.asarray(beta_out).reshape(-1)[0]),
        )
    nc = _CACHE[key]

    in_maps = _prep_inputs(
        x, W1, Wrec, W2, W3, alpha1, rho1, beta_a1, alpha2, rho2, beta_a2, beta_out
    )
    res = run_bass_kernel_spmd(nc, in_maps, list(range(N_CORES)), trace=_trace)

    out = np.empty((B, N_OUT), np.float32)
    for c in range(N_CORES):
        out[c * BC : (c + 1) * BC] = np.asarray(res.results[c]["out"]).T
    if _trace:
        return out, res
    return out


# revision 26
# speedup vs baseline: 1.0790x; 1.0790x over previous
"""Trainium2 Bass kernel for a recurrent adaptive-LIF SNN.

Network (per reference):
    B=1024, T=100, n_in=120, h1=512, h2=256, n_out=35
    per step t:
        cur1 = x_t @ W1.T + s1 @ Wrec.T
        a1' = rho1*a1 + (1-rho1)*s1
        v1' = alpha1*v1*(1-s1) + (1-alpha1)*cur1
        s1' = (v1' - (1 + beta_a1*a1') > 0)
        cur2 = s1' @ W2.T ; same LIF for layer 2
        vo' = beta_out*vo + (1-beta_out)*(s2' @ W3.T)
    out = mean_t vo(t)

Sharding: data-parallel over batch across 8 cores (128 batch/core),
weights replicated; the sequential T loop is local per core.

Layout: feature-major [feature -> partitions, batch -> free].

Layer 1 (shifted P1 := v1'-1; scaled copy cp1 := P1/cb1, cb1=beta_a1(1-rho1)):
    p1psum = W1a@[x;1] + WrecF@s1 + a1*cb1*(cp1_prev - r1_prev)   (diag matmuls)
    cp1    = ACT(p1psum, scale=1/cb1)        [= P1/cb1]
    u1'    = rho1*u1 + s1                    [STT, GPSIMD]
    s1'    = (u1' < cp1)                     [DVE is_lt  <=> cb1*u1' < P1]
    r1'    = s1' * cp1                       [DVE mult]
  since -a1*q1 = -a1*(s1-1)*P1 = a1*cb1*(cp1 - r1).  W1a has a const row
  (a1-1); WrecF = ((1-a1)Wrec).T - a1*I runs as fp8e4 DoubleRow with a 2^7
  weight scale balanced by fp8 spike mirrors sf = 2^-7*s1.
  Init: cp1_init = -1/cb1, r1_init = 0  (reproduces q1_init = 1).

Layer 2 (scaled Pt2 := v2/cb2; psum tracks Pt2 - a2/cb2):
    p2psum = W2s@s1' + a2*(cp2_prev - r2_prev) - (a2/cb2)*s2_prev  (diags)
    cp2    = ACT(p2psum, bias=(a2-1)/cb2)    [= Pt2 - 1/cb2]
    W2t'   = rho2*W2t + s2                   [STT, DVE]
    s2'    = (W2t' < cp2)                    [DVE]
    r2'    = s2' * cp2                       [GPSIMD mult]
  W2s = 2^7*((1-a2)/cb2 * W2).T as fp8e4 DoubleRow over the s1 mirrors.
  Init r2_init = 1/cb2, cp2_init = 0.

Output (closed form, no integrator state, no epilogue GEMM):
    out = sum_t (c_t*W3) @ s2(t),  c_t = (1 - beta_out^(T-t))/T
  accumulated directly in a persistent PSUM region with per-step
  pre-scaled stationary weights.

Engine split per step (cost-model balanced; the HW ISA rejects
TensorScalarPtr on Pool, so the STT-ish updates stay on DVE):
  PE    : W1 prefetch, fp8-DR Wrec + W2, diag compensations, out accumulation
  ACT   : the three scaled PSUM->SBUF copies (cp1 a/b halves, cp2)
  DVE   : spike compares, fp8 mirrors, u1 update as tensor_scalar(4x mode)
          + add(2x mode) [cheaper than the modeless STT], W2t STT
  GPSIMD: the reset products r = s*cp (full-step slack hides Q7 latency)
"""

import sys
import numpy as np

sys.path.insert(0, "/opt/trn_rl_repo")

import ml_dtypes

bf16 = ml_dtypes.bfloat16
f8e4 = ml_dtypes.float8_e4m3

# Problem constants (hardcoded per contract)
B, T, N_IN, H1, H2, N_OUT = 1024, 100, 120, 512, 256, 35
N_CORES = 8
BC = B // N_CORES  # 128 batch per core
C1 = H1 // 128     # 4 feature chunks, layer 1
C2 = H2 // 128     # 2 feature chunks, layer 2
K1 = N_IN + 1      # x augmented with a constant-one row

FP8_SHIFT = 7      # wrec8/w28 = 2^7 * W, fp8 spike mirror value = 2^-7

_CACHE = {}


def _build(alpha1, rho1, beta_a1, alpha2, rho2, beta_a2, beta_out):
    import concourse.bacc as bacc
    import concourse.mybir as mybir
    import concourse.tile as tile
    from concourse.alu_op_type import AluOpType

    fp32 = mybir.dt.float32
    bft = mybir.dt.bfloat16
    f8t = mybir.dt.float8e4
    A = AluOpType
    IDENT = mybir.ActivationFunctionType.Identity
    DR = mybir.MatmulPerfMode.DoubleRow

    a1 = float(alpha1)
    a2 = float(alpha2)
    cb1 = float(beta_a1 * (1.0 - rho1))
    cb2 = float(beta_a2 * (1.0 - rho2))
    sfp8 = float(2.0 ** -FP8_SHIFT)

    nc = bacc.Bacc()

    x_d = nc.declare_dram_parameter("x", [K1, T, BC], bft, isOutput=False)
    w1_d = nc.declare_dram_parameter("w1s", [K1, C1, 128], bft, isOutput=False)
    wr_d = nc.declare_dram_parameter(
        "wrec8", [128, 2, C1, 2, 128], f8t, isOutput=False
    )
    w2_d = nc.declare_dram_parameter(
        "w28", [128, 2, C2, 2, 128], f8t, isOutput=False
    )
    w3_d = nc.declare_dram_parameter("w3c", [128, C2, T, N_OUT], bft, isOutput=False)
    # diagonal compensation matrices
    dg_d = nc.declare_dram_parameter("diags", [128, 7, 128], bft, isOutput=False)
    out_d = nc.declare_dram_parameter("out", [N_OUT, BC], fp32, isOutput=True)

    XCH = 10  # x preload chunks
    TP = T // XCH

    with tile.TileContext(nc) as tc:
        with (
            tc.tile_pool(name="wpool", bufs=1) as wpool,
            tc.tile_pool(name="xpool", bufs=1) as xpool,
            tc.tile_pool(name="st1", bufs=12) as st1,
            tc.tile_pool(name="st2", bufs=8) as st2,
            tc.tile_pool(name="cp", bufs=12) as cpp,
            tc.tile_pool(name="sf", bufs=8) as sfp,
            tc.tile_pool(name="ps1a", bufs=2, space="PSUM") as ps1a,
            tc.tile_pool(name="ps1b", bufs=2, space="PSUM") as ps1b,
            tc.tile_pool(name="ps2", bufs=1, space="PSUM") as ps2,
            tc.tile_pool(name="psW", bufs=1, space="PSUM") as psW,
            tc.tile_pool(name="psO", bufs=1, space="PSUM") as psO,
        ):
            # ---- resident weights ----
            w1_s = wpool.tile([K1, C1, 128], bft, tag="w1")
            nc.sync.dma_start(w1_s[:], w1_d[:])
            wr_s = wpool.tile([128, 2, C1, 2, 128], f8t, tag="wr")
            nc.sync.dma_start(wr_s[:], wr_d[:])
            w2_s = wpool.tile([128, 2, C2, 2, 128], f8t, tag="w2")
            nc.sync.dma_start(w2_s[:], w2_d[:])
            w3_s = wpool.tile([128, C2, T, N_OUT], bft, tag="w3")
            nc.sync.dma_start(w3_s[:], w3_d[:])
            dg_s = wpool.tile([128, 7, 128], bft, tag="dg")
            nc.sync.dma_start(dg_s[:], dg_d[:])
            # diag slots: 0:+a1*cb1  1:-a1*cb1  2:+a2  3:-a2  4:-a2/cb2
            #             5:rho2     6:identity
            PD1, ND1, PD2, ND2, SD2, RD2, ID = (dg_s[:, i, :] for i in range(7))
            # per-partition bias column for the layer-2 scaled copy
            b2_s = wpool.tile([128, 1], fp32, tag="b2")
            nc.vector.memset(b2_s[:], (a2 - 1.0) / cb2)

            # ---- x preload in chunks ----
            x_tiles = []
            for i in range(XCH):
                xt = xpool.tile([K1, TP, BC], bft, tag=f"x{i}")
                nc.sync.dma_start(xt[:], x_d[:, i * TP : (i + 1) * TP, :])
                x_tiles.append(xt)

            # ---- initial states ----
            s1 = st1.tile([128, C1, BC], bft, tag="s1")
            nc.vector.memset(s1[:], 0.0)
            u1 = st1.tile([128, C1, BC], bft, tag="u1")
            nc.vector.memset(u1[:], 0.0)
            r1 = st1.tile([128, C1, BC], bft, tag="r1")
            nc.vector.memset(r1[:], 0.0)
            cpa = cpp.tile([128, 2, BC], bft, tag="cpa")
            nc.vector.memset(cpa[:], -1.0 / cb1)
            cpb = cpp.tile([128, 2, BC], bft, tag="cpb")
            nc.vector.memset(cpb[:], -1.0 / cb1)
            sfa = sfp.tile([128, 2, BC], f8t, tag="sfa")
            nc.vector.memset(sfa[:], 0.0)
            sfb = sfp.tile([128, 2, BC], f8t, tag="sfb")
            nc.vector.memset(sfb[:], 0.0)
            w2t = st2.tile([128, C2, BC], bft, tag="w2t")
            nc.vector.memset(w2t[:], 0.0)
            s2 = st2.tile([128, C2, BC], bft, tag="s2")
            nc.vector.memset(s2[:], 0.0)
            r2 = st2.tile([128, C2, BC], bft, tag="r2")
            nc.vector.memset(r2[:], 1.0 / cb2)
            cp2 = cpp.tile([128, C2, BC], bft, tag="cp2")
            nc.vector.memset(cp2[:], 0.0)

            # chunk -> s1n/u1n/r1 tile slot.  Bank A (p1a) holds chunks 0,2
            # (slots 0,1 -> copy cpa); bank B (p1b) holds chunks 1,3 (slots
            # 2,3 -> copy cpb).  This pairing lets each bank's reset half (r)
            # close its own recurrence without waiting for the other.
            CS = {0: 0, 2: 1, 1: 2, 3: 3}
            # bank -> (chunk in slot 0, chunk in slot 1)
            BANK_CHUNKS = {0: (0, 2), 1: (1, 3)}

            # ---- prologue: open the p1 banks for t=0.  Each bank is a
            # single accumulation group over [128, 2, BC]: the W1 prefetch
            # (slot-0 slice) carries start=True; slice writers use
            # skip_group_check; the full-tile ND1 carries stop.
            p1a = ps1a.tile([128, 2, BC], fp32, tag="p1a")
            p1b = ps1b.tile([128, 2, BC], fp32, tag="p1b")
            for m, ph in ((0, p1a), (1, p1b)):
                nc.tensor.matmul(
                    ph[:, 0, :], w1_s[:, m, :], x_tiles[0][:, 0, :],
                    start=True, stop=False,
                )

            def l1_bank(bank, ph, xsl_t):
                # close bank `bank` for this step: W1 slot-1 slice, full-tile
                # PD1 compensation, the four DR wrec slices, full-tile ND1
                # (stop).  All one PSUM group; no intra-bank region ordering.
                c_lo, c_hi = BANK_CHUNKS[bank]
                cph = cpa if bank == 0 else cpb
                rsl = r1[:, 2 * bank : 2 * bank + 2, :]
                nc.tensor.matmul(ph[:, 1, :], w1_s[:, c_hi, :], xsl_t,
                                 start=False, stop=False, skip_group_check=True)
                nc.tensor.matmul(ph[:], PD1, cph[:], start=False, stop=False,
                                 skip_group_check=True)
                for sl, m in ((0, c_lo), (1, c_hi)):
                    nc.tensor.matmul(ph[:, sl, :], wr_s[:, 0, m, :, :], sfa[:],
                                     start=False, stop=False, perf_mode=DR,
                                     skip_group_check=True)
                    nc.tensor.matmul(ph[:, sl, :], wr_s[:, 1, m, :, :], sfb[:],
                                     start=False, stop=False, perf_mode=DR,
                                     skip_group_check=True)
                nc.tensor.matmul(ph[:], ND1, rsl, start=False, stop=True,
                                 skip_group_check=True)

            def l2_state(tau):
                # L2 state updates for step tau.  w2t already holds W~2(tau)
                # (computed on PE+ACT during step tau).  The compare/reset run
                # on DVE right behind cp2; W~2(tau+1) = rho2*W~2 + s2 runs as
                # two full-tile diag matmuls + one ACT copy (a full step of
                # slack).
                nonlocal s2, r2
                s2n = st2.tile([128, C2, BC], bft, tag="s2")
                r2n = st2.tile([128, C2, BC], bft, tag="r2")
                pw = None
                nc.vector.tensor_tensor(s2n[:], w2t[:], cp2[:], A.is_lt)
                if tau < T - 1:
                    nc.gpsimd.tensor_tensor(r2n[:], s2n[:], cp2[:], A.mult)
                # output accumulation + the W~2 recurrence have nearly a full
                # step of slack -> deprioritized so they never clog the PE
                # queue ahead of cycle-critical work
                with tc.high_priority(offset=-1_000_000):
                    # out += (cw_tau * W3)[k] @ s2n[k]
                    for k in range(C2):
                        nc.tensor.matmul(
                            out_ps[:], w3_s[:, k, tau, :], s2n[:, k, :],
                            start=(tau == 0 and k == 0),
                            stop=(tau == T - 1 and k == C2 - 1),
                            skip_group_check=True,
                        )
                if tau < T - 1:
                    # normal priority: pw must close early so the w2tn copy
                    # runs in ACT's mid-step idle window
                    pw = psW.tile([128, C2, BC], fp32, tag="pw")
                    nc.tensor.matmul(pw[:], RD2, w2t[:], start=True, stop=False)
                    nc.tensor.matmul(pw[:], ID, s2n[:], start=False, stop=True)
                s2, r2 = s2n, r2n
                return pw if tau < T - 1 else None

            out_ps = psO.tile([N_OUT, BC], fp32, tag="ops")

            u1n = u1
            for t in range(T):
                # ---- L2 state updates for step t-1 (cmp2 heads the DVE
                # queue: its input cp2(t-1) is ready at step start) ----
                pw_t = l2_state(t - 1) if t >= 1 else None

                # ---- u1 adaptation, part 1: u1t(t+1) = rho1*u1n(t); input
                # ready since the tail of step t-1, fills early-step DVE idle
                u1t_n = st1.tile([128, C1, BC], bft, tag="u1t")
                nc.vector.tensor_scalar(u1t_n[:], u1n[:], float(rho1), None, A.mult)

                # ---- finish p1(t): single accumulation group per bank ----
                xsl_t = x_tiles[t // TP][:, t % TP, :]
                with tc.high_priority(offset=500_000):
                    l1_bank(0, p1a, xsl_t)
                    l1_bank(1, p1b, xsl_t)

                # ---- scaled PSUM -> SBUF copies of P1 ----
                cpa_n = cpp.tile([128, 2, BC], bft, tag="cpa")
                nc.scalar.activation(cpa_n[:], p1a[:], IDENT, scale=1.0 / cb1)
                cpb_n = cpp.tile([128, 2, BC], bft, tag="cpb")
                nc.scalar.activation(cpb_n[:], p1b[:], IDENT, scale=1.0 / cb1)

                # ---- layer-1 spike compares + fp8 mirrors on DVE (per bank
                # half so each bank's recurrence closes independently).
                # Reset products: bank A's goes to GPSIMD (it closes early,
                # so Pool's latency hides); bank B's stays on DVE right
                # behind its mirror so the B-side cycle never waits on Pool.
                s1n = st1.tile([128, C1, BC], bft, tag="s1")
                r1n = st1.tile([128, C1, BC], bft, tag="r1")
                sfa_n = sfp.tile([128, 2, BC], f8t, tag="sfa")
                sfb_n = sfp.tile([128, 2, BC], f8t, tag="sfb")
                nc.vector.tensor_tensor(
                    s1n[:, 0:2, :], u1n[:, 0:2, :], cpa_n[:], A.is_lt
                )
                nc.vector.tensor_scalar(sfa_n[:], s1n[:, 0:2, :], sfp8, None, A.mult)
                if t + 1 < T:
                    nc.gpsimd.tensor_tensor(
                        r1n[:, 0:2, :], s1n[:, 0:2, :], cpa_n[:], A.mult
                    )
                nc.vector.tensor_tensor(
                    s1n[:, 2:4, :], u1n[:, 2:4, :], cpb_n[:], A.is_lt
                )
                nc.vector.tensor_scalar(sfb_n[:], s1n[:, 2:4, :], sfp8, None, A.mult)
                if t + 1 < T:
                    nc.vector.tensor_tensor(
                        r1n[:, 2:4, :], s1n[:, 2:4, :], cpb_n[:], A.mult
                    )

                # ---- u1 adaptation, part 2: u1n(t+1) = u1t + s1n(t) ----
                u1n_n = st1.tile([128, C1, BC], bft, tag="u1")
                nc.vector.tensor_tensor(u1n_n[:], u1t_n[:], s1n[:], A.add)

                # ---- W1 prefetch into next step's p1 banks (slot-0 slice
                # carries the group start) ----
                if t + 1 < T:
                    p1a_n = ps1a.tile([128, 2, BC], fp32, tag="p1a")
                    p1b_n = ps1b.tile([128, 2, BC], fp32, tag="p1b")
                    xsl = x_tiles[(t + 1) // TP][:, (t + 1) % TP, :]
                    nc.tensor.matmul(p1a_n[:, 0, :], w1_s[:, 0, :], xsl,
                                     start=True, stop=False)
                    nc.tensor.matmul(p1b_n[:, 0, :], w1_s[:, 1, :], xsl,
                                     start=True, stop=False)
                else:
                    p1a_n = p1b_n = None

                # ---- layer 2: p2 = W2s@s1' (fp8 DR over the mirrors) +
                # diag compensations; one accumulation group, full-tile
                # diags + DR slice writes ----
                p2 = ps2.tile([128, C2, BC], fp32, tag="p2")
                nc.tensor.matmul(p2[:], PD2, cp2[:], start=True, stop=False)
                nc.tensor.matmul(p2[:], SD2, s2[:], start=False, stop=False,
                                 skip_group_check=True)
                for m in range(C2):
                    nc.tensor.matmul(p2[:, m, :], w2_s[:, 0, m, :, :], sfa_n[:],
                                     start=False, stop=False, perf_mode=DR,
                                     skip_group_check=True)
                    nc.tensor.matmul(p2[:, m, :], w2_s[:, 1, m, :, :], sfb_n[:],
                                     start=False, stop=False, perf_mode=DR,
                                     skip_group_check=True)
                nc.tensor.matmul(p2[:], ND2, r2[:], start=False, stop=True,
                                 skip_group_check=True)

                # ---- tail copies: W~2 then cp2 on ACT, after the
                # cycle-critical cp1 copies in queue order ----
                if pw_t is not None:
                    w2tn = cpp.tile([128, C2, BC], bft, tag="w2t")
                    nc.scalar.activation(w2tn[:], pw_t[:], IDENT)
                    w2t = w2tn
                cp2_n = cpp.tile([128, C2, BC], bft, tag="cp2")
                nc.scalar.activation(cp2_n[:], p2[:], IDENT, bias=b2_s[:])

                s1, r1, u1n = s1n, r1n, u1n_n
                cpa, cpb, sfa, sfb = cpa_n, cpb_n, sfa_n, sfb_n
                cp2 = cp2_n
                p1a, p1b = p1a_n, p1b_n

            # ---- epilogue: last L2 state (includes final out term), then
            # writeback ----
            l2_state(T - 1)
            outf = cpp.tile([N_OUT, BC], fp32, tag="outf")
            nc.vector.tensor_scalar(outf[:], out_ps[:], 1.0, None, A.mult)
            nc.sync.dma_start(out_d[:], outf[:])

    nc.compile()
    return nc


def _prep_inputs(x, W1, Wrec, W2, W3, alpha1, rho1, beta_a1, alpha2, rho2, beta_a2, beta_out):
    a1 = float(np.asarray(alpha1).reshape(-1)[0])
    a2 = float(np.asarray(alpha2).reshape(-1)[0])
    r1 = float(np.asarray(rho1).reshape(-1)[0])
    r2 = float(np.asarray(rho2).reshape(-1)[0])
    ba1 = float(np.asarray(beta_a1).reshape(-1)[0])
    ba2 = float(np.asarray(beta_a2).reshape(-1)[0])
    bo = float(np.asarray(beta_out).reshape(-1)[0])
    cb1 = ba1 * (1.0 - r1)
    cb2 = ba2 * (1.0 - r2)

    w1s = ((1.0 - np.asarray(alpha1, np.float32)[:, None]) * np.asarray(W1, np.float32)).T
    wrs = ((1.0 - np.asarray(alpha1, np.float32)[:, None]) * np.asarray(Wrec, np.float32)).T
    w2s = (((1.0 - np.asarray(alpha2, np.float32)[:, None]) / cb2) * np.asarray(W2, np.float32)).T
    w3s = np.asarray(W3, np.float32).T

    # layer-1 shift folds:  WrecF = wrs - a1*I ; W1 gains const row (a1-1)
    wrs = wrs - a1 * np.eye(H1, dtype=np.float32)
    w1aug = np.concatenate(
        [w1s, np.full((1, H1), a1 - 1.0, np.float32)], axis=0
    )  # [121, 512]

    w1_a = np.ascontiguousarray(w1aug.reshape(K1, C1, 128)).astype(bf16)

    def pack_dr(w, cout):
        # w [H1, cout*128]: dr[part, pair, m, i, col] =
        #   2^7 * w[k(pair,i)*128+part, m*128+col], k(pair,i) = pair + 2*i
        # so DR pair 0 eats chunks (0,2) = spike mirror sfa and pair 1 eats
        # (1,3) = sfb.
        w4 = w.reshape(C1, 128, cout, 128)  # [k, part, m, col]
        w8 = (2.0 ** FP8_SHIFT) * w4.reshape(2, 2, 128, cout, 128)  # [i, pair, ...]
        return np.ascontiguousarray(w8.transpose(2, 1, 3, 0, 4)).astype(f8e4)

    wr_a = pack_dr(wrs, C1)
    w2_a = pack_dr(w2s, C2)

    # per-step output weights c_t = (1 - beta^(T-t))/T folded into W3
    cw = np.array([(1.0 - bo ** (T - t)) / T for t in range(T)], np.float32)
    w3_a = np.ascontiguousarray(
        w3s.reshape(C2, 128, N_OUT)[:, None, :, :].transpose(1, 0, 2, 3)
        * cw[:, None, None, None]
    )  # [T, C2, 128, N_OUT]
    w3_a = np.ascontiguousarray(w3_a.transpose(2, 1, 0, 3)).astype(bf16)
    # -> [128, C2, T, N_OUT]

    eye = np.eye(128, dtype=np.float32)
    diags = np.stack([
        (a1 * cb1) * eye,     # PD1
        (-a1 * cb1) * eye,    # ND1
        a2 * eye,             # PD2
        (-a2) * eye,          # ND2
        (-a2 / cb2) * eye,    # SD2
        r2 * eye,             # RD2
        eye,                  # ID
    ], axis=1).astype(bf16)   # [128, 7, 128]

    shared = dict(w1s=w1_a, wrec8=wr_a, w28=w2_a, w3c=w3_a, diags=diags)
    in_maps = []
    for c in range(N_CORES):
        xc = np.asarray(x[c * BC : (c + 1) * BC], np.float32)  # [BC, T, N_IN]
        xfm = xc.transpose(2, 1, 0)  # [N_IN, T, BC]
        xaug = np.concatenate([xfm, np.ones((1, T, BC), np.float32)], axis=0)
        in_maps.append(dict(x=np.ascontiguousarray(xaug).astype(bf16), **shared))
    return in_maps


def kernel(
    x, W1, Wrec, W2, W3,
    alpha1, rho1, beta_a1, alpha2, rho2, beta_a2, beta_out,
    _trace=False,
):
    from concourse.bass_utils import run_bass_kernel_spmd

    key = "nc"
    if key not in _CACHE:
        _CACHE[key] = _build(
            float(np.asarray(alpha1).reshape(-1)[0]),
            float(np.asarray(rho1).reshape(-1)[0]),
            float(np.asarray(beta_a1).reshape(-1)[0]),
            float(np.asarray(alpha2).reshape(-1)[0]),
            float(np.asarray(rho2).reshape(-1)[0]),
            float(np.asarray(beta_a2).reshape(-1)[0]),
            float(np.asarray(beta_out).reshape(-1)[0]),
        )
    nc = _CACHE[key]

    in_maps = _prep_inputs(
        x, W1, Wrec, W2, W3, alpha1, rho1, beta_a1, alpha2, rho2, beta_a2, beta_out
    )
    res = run_bass_kernel_spmd(nc, in_maps, list(range(N_CORES)), trace=_trace)

    out = np.empty((B, N_OUT), np.float32)
    for c in range(N_CORES):
        out[c * BC : (c + 1) * BC] = np.asarray(res.results[c]["out"]).T
    if _trace:
        return out, res
    return out
